# revision 1
# baseline (speedup 1.0000x reference)
"""Trainium2 Bass kernel for nn_Block_33328946217681 (dual-stream dense
transformer: 4x [self-attn + MLP] on two streams, then one cross-attn +
MLP exchange between streams).

Sharding: 8 cores, core 2b owns x[b], core 2b+1 owns y[b] (B=4).  Each core
runs the self-block stack on its own stream, then the pair (2b, 2b+1)
exchanges states with a pairwise AllReduce (partner = sum - own) and runs
the final cross-attention block.  Only the last loop iteration's cross
output is live in the reference, so earlier cross blocks are skipped.

Precision: matmul operands bf16 (weights pre-cast on host), fp32 residual
stream, fp32 PSUM accumulation, fp32 softmax statistics.
"""

import numpy as np
import ml_dtypes

import concourse.bass as bass
import concourse.bacc as bacc
import concourse.tile as tile
from concourse import mybir
from concourse.bass_utils import run_bass_kernel_spmd

BF16 = mybir.dt.bfloat16
F32 = mybir.dt.float32
F32R = mybir.dt.float32r
AF = mybir.ActivationFunctionType
ALU = mybir.AluOpType

B, N, C = 4, 512, 1024
H, D = 16, 64
HID = 4 * C
P = 128
NT = N // P      # 4 token chunks
CCH = C // P     # 8 channel chunks
HC = HID // P    # 32 hidden chunks
PAIRS = H // 2   # 8 head pairs
EPS = 1e-5
N_CORES = 8
REPLICA_GROUPS = [[0, 1], [2, 3], [4, 5], [6, 7]]

_cache = {}


def _layernorm(nc, pools, x_state, out_bf, g_tile, b_tile):
    """out_bf[P,NT,C] (bf16) = LN(x_state[P,NT,C] fp32) over C."""
    sb, ps = pools["sb"], pools["ps"]
    eps_t = pools["eps"]
    for t in range(NT):
        stats = sb.tile([P, 2, 6], F32, tag="lnstats", bufs=4, name=f"lnst{t}")
        xg = x_state[:, t, :].rearrange("p (b f) -> p b f", f=512)
        for g in range(2):
            nc.vector.bn_stats(stats[:, g, :], xg[:, g, :])
        mv = sb.tile([P, 2], F32, tag="lnmv", bufs=4, name=f"lnmv{t}")
        nc.vector.bn_aggr(mv[:], stats[:])
        rstd = sb.tile([P, 1], F32, tag="lnrstd", bufs=4, name=f"lnrs{t}")
        # rstd = exp(-0.5 * ln(var + eps)); stays in the exp/ln ACT table set
        nc.scalar.activation(rstd[:], mv[:, 1:2], AF.Ln, bias=eps_t[:])
        nc.scalar.activation(rstd[:], rstd[:], AF.Exp, scale=-0.5)
        nc.vector.tensor_scalar(
            out=out_bf[:, t, :], in0=x_state[:, t, :],
            scalar1=mv[:, 0:1], scalar2=rstd[:],
            op0=ALU.subtract, op1=ALU.mult)
        if g_tile is not None:
            nc.vector.tensor_mul(out_bf[:, t, :], out_bf[:, t, :], g_tile[:])
        if b_tile is not None:
            nc.vector.tensor_add(out_bf[:, t, :], out_bf[:, t, :], b_tile[:])


def _transpose(nc, pools, src_bf, dst_bf, id_bf):
    """dst_bf[P,CCH,N] = transpose of src_bf[P,NT,C] (channel-major)."""
    ps = pools["ps"]
    for t in range(NT):
        for c in range(CCH):
            pst = ps.tile([P, P], BF16, tag="ps_acc", bufs=4,
                          name=f"pstr{t}_{c}")
            nc.tensor.transpose(pst[:], src_bf[:, t, c * P:(c + 1) * P],
                                id_bf[:])
            nc.vector.tensor_copy(dst_bf[:, c, t * P:(t + 1) * P], pst[:])


def _attention(nc, pools, qT, kvT, kv_nat, ot, consts, self_mode):
    """ot[P,CCH,N] (bf16) = per-head softmax(qk/8) @ v, heads = channel dim.

    qT/kvT: [P,CCH,N] bf16 transposed normed activations (channel on part.)
    kv_nat: [P,NT,C]  bf16 normed activations (token on partitions)
    """
    sb, ps = pools["sb"], pools["ps"]
    id_f32 = consts["id_f32"]

    r_all = None
    ps_rt = None
    if self_mode:
        r_all = sb.tile([P, NT, H], F32, tag="r_all", bufs=2, name="r_all")
    else:
        ps_rt = ps.tile([16, N], F32, tag="ps_rt", bufs=1, name="ps_rt")
    rt = sb.tile([16, N], F32R, tag="rt", bufs=2, name="rt")

    n_denom = 0
    for j in range(PAIRS):
        ha, hb = 2 * j, 2 * j + 1
        e_a, e_b = [], []
        # scores S^T chunks + exp (row-packed head pair)
        for sc in range(NT):
            ssl = slice(sc * P, (sc + 1) * P)
            psa = ps.tile([P, N], F32, tag="ps_short", bufs=3,
                          name=f"psa{j}_{sc}")
            psb = ps.tile([P, N], F32, tag="ps_short", bufs=3,
                          name=f"psb{j}_{sc}")
            nc.tensor.matmul(psa[:], lhsT=kvT[0:64, j, ssl],
                             rhs=qT[0:64, j, :], start=True, stop=True,
                             tile_position=(0, 0))
            nc.tensor.matmul(psb[:], lhsT=kvT[64:128, j, ssl],
                             rhs=qT[64:128, j, :], start=True, stop=True,
                             tile_position=(64, 0))
            ea = sb.tile([P, N], BF16, tag="eh", bufs=32, name=f"ea{j}_{sc}")
            eb = sb.tile([P, N], BF16, tag="eh", bufs=32, name=f"eb{j}_{sc}")
            if self_mode:
                # symmetric E: free-dim accumulation gives the softmax denom
                nc.scalar.activation(ea[:], psa[:], AF.Exp, scale=0.125,
                                     accum_out=r_all[:, sc, ha:ha + 1])
                nc.scalar.activation(eb[:], psb[:], AF.Exp, scale=0.125,
                                     accum_out=r_all[:, sc, hb:hb + 1])
            else:
                nc.scalar.activation(ea[:], psa[:], AF.Exp, scale=0.125)
                nc.scalar.activation(eb[:], psb[:], AF.Exp, scale=0.125)
            e_a.append(ea)
            e_b.append(eb)

        if not self_mode:
            # denominators: rows of ps_rt accumulate sum_s E^T[s, n] per head
            sel = consts["sel"]
            for sc in range(NT):
                for hh, ee in ((ha, e_a[sc]), (hb, e_b[sc])):
                    nc.tensor.matmul(
                        ps_rt[:], lhsT=sel[:, hh, :], rhs=ee[:],
                        start=(n_denom == 0),
                        stop=(n_denom == 2 * PAIRS * NT - 1),
                        tile_position=(0, 0))
                    n_denom += 1

        # AV: U^T accumulate over s chunks, col-packed head pair
        psu = ps.tile([P, N], F32, tag="ps_acc", bufs=4, name=f"psu{j}")
        for sc in range(NT):
            nc.tensor.matmul(psu[0:64, :],
                             lhsT=kv_nat[:, sc, ha * D:(ha + 1) * D],
                             rhs=e_a[sc][:], start=(sc == 0),
                             stop=(sc == NT - 1), tile_position=(0, 0))
            nc.tensor.matmul(psu[64:128, :],
                             lhsT=kv_nat[:, sc, hb * D:(hb + 1) * D],
                             rhs=e_b[sc][:], start=(sc == 0),
                             stop=(sc == NT - 1), tile_position=(0, 64))
        # unnormalized U^T into the output tile (bf16)
        nc.vector.tensor_copy(ot[:, j, :], psu[:])

    # reciprocal denominators, laid out [16 heads, N]
    if self_mode:
        for sc in range(NT):
            pst = ps.tile([16, P], F32, tag="ps_short", bufs=3,
                          name=f"psrt{sc}")
            nc.tensor.transpose(pst[:], r_all[:, sc, :], id_f32[:])
            nc.vector.tensor_copy(rt[:, sc * P:(sc + 1) * P], pst[:])
        with nc.allow_low_precision(reason="softmax denom recip in f32r"):
            nc.vector.reciprocal(rt[:], rt[:])
    else:
        with nc.allow_low_precision(reason="softmax denom recip in f32r"):
            nc.vector.reciprocal(rt[:], ps_rt[:])

    # normalize: broadcast recip rows over head partitions via K=16 matmul
    bmat = consts["bmat"]
    for j in range(PAIRS):
        psc = ps.tile([P, N], F32, tag="ps_short", bufs=3, name=f"psbc{j}")
        nc.tensor.matmul(psc[:], lhsT=bmat[:, j * P:(j + 1) * P],
                         rhs=rt[:], start=True, stop=True,
                         tile_position=(0, 0))
        nc.vector.tensor_mul(ot[:, j, :], ot[:, j, :], psc[:])


def _proj_residual(nc, pools, ot, w_sb, x_state, bias_tile):
    """x_state += ot.T @ w  (w_sb: [P,CCH,C] bf16)."""
    ps = pools["ps"]
    for t in range(NT):
        for co in range(2):
            cosl = slice(co * 512, (co + 1) * 512)
            psm = ps.tile([P, 512], F32, tag="ps_acc", bufs=4,
                          name=f"pspj{t}_{co}")
            for c in range(CCH):
                nc.tensor.matmul(psm[:], lhsT=ot[:, c, t * P:(t + 1) * P],
                                 rhs=w_sb[:, c, cosl], start=(c == 0),
                                 stop=(c == CCH - 1))
            nc.vector.tensor_add(x_state[:, t, cosl], x_state[:, t, cosl],
                                 psm[:])
            if bias_tile is not None:
                nc.vector.tensor_add(x_state[:, t, cosl],
                                     x_state[:, t, cosl], bias_tile[:, cosl])


def _mlp(nc, pools, x_state, consts, flags):
    """x_state += fc2(gelu(fc1(LN2(x_state))))."""
    sb, ps = pools["sb"], pools["ps"]
    x2n = sb.tile([P, NT, C], BF16, tag="n_bf", bufs=2, name="x2n")
    _layernorm(nc, pools, x_state, x2n, consts.get("g2t"), consts.get("b2t"))
    x2T = sb.tile([P, CCH, N], BF16, tag="nT", bufs=2, name="x2T")
    _transpose(nc, pools, x2n, x2T, consts["id_bf"])

    fc1w, fc2w_dram = consts["fc1w"], consts["fc2w_dram"]
    fc1b = consts.get("fc1bt")
    hacts = []
    for ht in range(HC):
        psh = ps.tile([P, N], F32, tag="ps_acc", bufs=4, name=f"psh{ht}")
        for c in range(CCH):
            nc.tensor.matmul(psh[:], lhsT=fc1w[:, c, ht * P:(ht + 1) * P],
                             rhs=x2T[:, c, :], start=(c == 0),
                             stop=(c == CCH - 1))
        hact = sb.tile([P, N], BF16, tag="eh", bufs=32, name=f"hact{ht}")
        if fc1b is not None:
            nc.scalar.activation(hact[:], psh[:], AF.Gelu,
                                 bias=fc1b[:, ht:ht + 1])
        else:
            nc.scalar.activation(hact[:], psh[:], AF.Gelu)
        hacts.append(hact)

    fc2b = consts.get("fc2bt")
    for co in range(2):
        cosl = slice(co * 512, (co + 1) * 512)
        psms = [ps.tile([P, 512], F32, tag="ps_acc", bufs=4,
                        name=f"psm2_{co}_{t}") for t in range(NT)]
        for hc in range(HC):
            wt = sb.tile([P, 512], BF16, tag="fc2w", bufs=6,
                         name=f"f2w{co}_{hc}")
            nc.sync.dma_start(wt[:], fc2w_dram[hc * P:(hc + 1) * P, cosl])
            for t in range(NT):
                nc.tensor.matmul(psms[t][:],
                                 lhsT=hacts[hc][:, t * P:(t + 1) * P],
                                 rhs=wt[:], start=(hc == 0),
                                 stop=(hc == HC - 1))
        for t in range(NT):
            nc.vector.tensor_add(x_state[:, t, cosl], x_state[:, t, cosl],
                                 psms[t][:])
            if fc2b is not None:
                nc.vector.tensor_add(x_state[:, t, cosl],
                                     x_state[:, t, cosl], fc2b[:, cosl])


def _block(nc, pools, x_state, consts, flags, kv_state=None):
    """One transformer block.  kv_state=None -> self-attn on x_state."""
    sb = pools["sb"]
    xn = sb.tile([P, NT, C], BF16, tag="n_bf", bufs=2, name="xn")
    _layernorm(nc, pools, x_state, xn, consts.get("g1t"), consts.get("b1t"))
    xnT = sb.tile([P, CCH, N], BF16, tag="nT", bufs=2, name="xnT")
    _transpose(nc, pools, xn, xnT, consts["id_bf"])

    if kv_state is None:
        kv_n, kv_T, self_mode = xn, xnT, True
    else:
        kv_n = sb.tile([P, NT, C], BF16, tag="n_bf", bufs=2, name="pn")
        _layernorm(nc, pools, kv_state, kv_n, consts.get("g1t"),
                   consts.get("b1t"))
        kv_T = sb.tile([P, CCH, N], BF16, tag="nT", bufs=2, name="pnT")
        _transpose(nc, pools, kv_n, kv_T, consts["id_bf"])
        self_mode = False

    ot = sb.tile([P, CCH, N], BF16, tag="ot", bufs=1, name="ot")
    _attention(nc, pools, xnT, kv_T, kv_n, ot, consts, self_mode)
    _proj_residual(nc, pools, ot, consts["projw"], x_state,
                   consts.get("projbt"))
    _mlp(nc, pools, x_state, consts, flags)


def _build(n_self, flags, exchange=True):
    """flags: dict of bools: g1,b1,g2,b2,projb,fc1b,fc2b nontrivial."""
    nc = bacc.Bacc("TRN2", target_bir_lowering=False, debug=False,
                   num_devices=N_CORES)

    own_d = nc.dram_tensor("own", [P, NT, C], F32, kind="ExternalInput").ap()
    projw_d = nc.dram_tensor("projw", [P, CCH, C], BF16,
                             kind="ExternalInput").ap()
    fc1w_d = nc.dram_tensor("fc1w", [P, CCH, HID], BF16,
                            kind="ExternalInput").ap()
    fc2w_d = nc.dram_tensor("fc2w", [HID, C], BF16, kind="ExternalInput").ap()
    idbf_d = nc.dram_tensor("id_bf", [P, P], BF16, kind="ExternalInput").ap()
    idf_d = nc.dram_tensor("id_f32", [P, P], F32, kind="ExternalInput").ap()
    sel_d = nc.dram_tensor("sel", [P, H, H], BF16, kind="ExternalInput").ap()
    bmat_d = nc.dram_tensor("bmat", [16, C], F32R, kind="ExternalInput").ap()
    extra_d = {}
    for nm, shape in (("g1", [C]), ("b1", [C]), ("g2", [C]), ("b2", [C]),
                      ("projb", [C]), ("fc2b", [C])):
        if flags[nm]:
            extra_d[nm] = nc.dram_tensor(nm, shape, F32,
                                         kind="ExternalInput").ap()
    if flags["fc1b"]:
        extra_d["fc1b"] = nc.dram_tensor("fc1b", [P, HC], F32,
                                         kind="ExternalInput").ap()
    out_d = nc.dram_tensor("out", [P, NT, C], F32, kind="ExternalOutput").ap()

    with tile.TileContext(nc) as tc:
        with tc.tile_pool(name="sb", bufs=1) as sb, \
             tc.tile_pool(name="ps", bufs=1, space="PSUM") as ps, \
             tc.tile_pool(name="dram", bufs=1, space="DRAM") as dram:
            pools = {"sb": sb, "ps": ps, "dram": dram}
            eps_t = sb.tile([P, 1], F32, tag="eps", name="eps_t")
            nc.vector.memset(eps_t[:], EPS)
            pools["eps"] = eps_t

            # persistent state + resident weights + constants
            x_state = sb.tile([P, NT, C], F32, tag="x_state", name="x_state")
            nc.sync.dma_start(x_state[:], own_d)
            projw = sb.tile([P, CCH, C], BF16, tag="projw", name="projw")
            nc.sync.dma_start(projw[:], projw_d)
            fc1w = sb.tile([P, CCH, HID], BF16, tag="fc1w", name="fc1w")
            nc.sync.dma_start(fc1w[:], fc1w_d)
            id_bf = sb.tile([P, P], BF16, tag="id_bf", name="id_bf")
            nc.sync.dma_start(id_bf[:], idbf_d)
            id_f32 = sb.tile([P, P], F32, tag="id_f32", name="id_f32")
            nc.sync.dma_start(id_f32[:], idf_d)
            sel = sb.tile([P, H, H], BF16, tag="sel", name="sel")
            nc.sync.dma_start(sel[:], sel_d)
            bmat = sb.tile([16, C], F32R, tag="bmat", name="bmat")
            nc.sync.dma_start(bmat[:], bmat_d)

            consts = {"id_bf": id_bf, "id_f32": id_f32, "sel": sel,
                      "bmat": bmat, "projw": projw, "fc1w": fc1w,
                      "fc2w_dram": fc2w_d}
            # optional gain/bias tiles
            for nm, key in (("g1", "g1t"), ("b1", "b1t"), ("g2", "g2t"),
                            ("b2", "b2t"), ("projb", "projbt"),
                            ("fc2b", "fc2bt")):
                if flags[nm]:
                    t_ = sb.tile([P, C], F32, tag=nm, name=nm + "t")
                    nc.sync.dma_start(t_[:],
                                      extra_d[nm].to_broadcast((P, C)))
                    consts[key] = t_
            if flags["fc1b"]:
                t_ = sb.tile([P, HC], F32, tag="fc1b", name="fc1bt")
                nc.sync.dma_start(t_[:], extra_d["fc1b"])
                consts["fc1bt"] = t_

            for _ in range(n_self):
                _block(nc, pools, x_state, consts, flags)

            # exchange states within the pair, then cross-attention block
            partner = sb.tile([P, NT, C], F32, tag="partner", name="partner")
            if exchange:
                b_in = dram.tile([P, NT, C], F32, name="b_in")
                b_out = dram.tile([P, NT, C], F32, name="b_out")
                nc.sync.dma_start(b_in[:], x_state[:])
                nc.gpsimd.collective_compute(
                    "AllReduce", ALU.add, replica_groups=REPLICA_GROUPS,
                    ins=[b_in.opt()], outs=[b_out.opt()])
                nc.sync.dma_start(partner[:], b_out[:])
                nc.vector.tensor_sub(partner[:], partner[:], x_state[:])
            else:
                nc.vector.tensor_copy(partner[:], x_state[:])

            _block(nc, pools, x_state, consts, flags, kv_state=partner)

            nc.sync.dma_start(out_d, x_state[:])
    nc.compile()
    return nc


def _get_nc(n_self, flags):
    key = (n_self, tuple(sorted(flags.items())))
    if key not in _cache:
        _cache[key] = _build(n_self, flags)
    return _cache[key]


def _nontrivial(a, val=0.0):
    return not np.allclose(np.asarray(a, np.float32), val, atol=0.0, rtol=0.0)


def kernel(**inputs):
    x = np.ascontiguousarray(np.asarray(inputs["x"], np.float32))
    y = np.ascontiguousarray(np.asarray(inputs["y"], np.float32))
    n1g, n1b = inputs["norm1_g"], inputs["norm1_b"]
    n2g, n2b = inputs["norm2_g"], inputs["norm2_b"]
    proj_w, proj_b = inputs["proj_w"], inputs["proj_b"]
    fc1_w, fc1_b = inputs["fc1_w"], inputs["fc1_b"]
    fc2_w, fc2_b = inputs["fc2_w"], inputs["fc2_b"]
    is_selfatt = int(np.asarray(inputs["is_selfatt"]))

    flags = {
        "g1": _nontrivial(n1g, 1.0), "b1": _nontrivial(n1b),
        "g2": _nontrivial(n2g, 1.0), "b2": _nontrivial(n2b),
        "projb": _nontrivial(proj_b), "fc1b": _nontrivial(fc1_b),
        "fc2b": _nontrivial(fc2_b),
    }
    n_self = 4 if is_selfatt else 0
    nc = _get_nc(n_self, flags)

    bf = ml_dtypes.bfloat16
    projw_h = np.ascontiguousarray(
        np.asarray(proj_w, np.float32).reshape(CCH, P, C).transpose(1, 0, 2)
    ).astype(bf)
    fc1w_h = np.ascontiguousarray(
        np.asarray(fc1_w, np.float32).reshape(CCH, P, HID).transpose(1, 0, 2)
    ).astype(bf)
    fc2w_h = np.ascontiguousarray(np.asarray(fc2_w, np.float32)).astype(bf)
    id_h = np.eye(P, dtype=np.float32)
    sel_h = np.zeros((P, H, H), np.float32)
    sel_h[:, np.arange(H), np.arange(H)] = 1.0
    sel_h = sel_h.astype(bf)
    bmat_h = np.zeros((16, C), np.float32)
    for j in range(PAIRS):
        bmat_h[2 * j, j * P:j * P + 64] = 1.0
        bmat_h[2 * j + 1, j * P + 64:(j + 1) * P] = 1.0

    base = {
        "projw": projw_h, "fc1w": fc1w_h, "fc2w": fc2w_h,
        "id_bf": id_h.astype(bf), "id_f32": id_h,
        "sel": sel_h, "bmat": bmat_h,
    }
    for nm, arr in (("g1", n1g), ("b1", n1b), ("g2", n2g), ("b2", n2b),
                    ("projb", proj_b), ("fc2b", fc2_b)):
        if flags[nm]:
            base[nm] = np.ascontiguousarray(np.asarray(arr, np.float32))
    if flags["fc1b"]:
        base["fc1b"] = np.ascontiguousarray(
            np.asarray(fc1_b, np.float32).reshape(HC, P).T)

    in_maps = []
    for core in range(N_CORES):
        bidx = core // 2
        own = x[bidx] if core % 2 == 0 else y[bidx]
        own_dev = np.ascontiguousarray(
            own.reshape(NT, P, C).transpose(1, 0, 2))
        m = dict(base)
        m["own"] = own_dev
        in_maps.append(m)

    res = run_bass_kernel_spmd(nc, in_maps, core_ids=list(range(N_CORES)))

    def unpack(core):
        o = np.asarray(res.results[core]["out"], np.float32)
        return o.transpose(1, 0, 2).reshape(N, C)

    x1 = np.stack([unpack(2 * b) for b in range(B)])
    y1 = np.stack([unpack(2 * b + 1) for b in range(B)])
    return (x1, y1)



# revision 12
# speedup vs baseline: 2.5159x; 2.5159x over previous
"""Trainium2 Bass kernel for nn_Block_33328946217681 (dual-stream dense
transformer: 4x [self-attn + MLP] on two streams, then one cross-attn +
MLP exchange between streams).

Sharding: 8 cores, core 2b owns x[b], core 2b+1 owns y[b] (B=4).  Each core
runs the self-block stack on its own stream, then the pair (2b, 2b+1)
exchanges states with a pairwise AllReduce (partner = sum - own) and runs
the final cross-attention block.  Only the last loop iteration's cross
output is live in the reference, so earlier cross blocks are skipped.

Precision: matmul operands bf16 (weights pre-cast on host), fp32 residual
stream, fp32 PSUM accumulation, fp32 softmax statistics.
"""

import numpy as np
import ml_dtypes

import concourse.bass as bass
import concourse.bacc as bacc
import concourse.tile as tile
from concourse import mybir
from concourse.bass_utils import run_bass_kernel_spmd

BF16 = mybir.dt.bfloat16
F32 = mybir.dt.float32
F32R = mybir.dt.float32r
AF = mybir.ActivationFunctionType
ALU = mybir.AluOpType

B, N, C = 4, 512, 1024
H, D = 16, 64
HID = 4 * C
P = 128
NT = N // P      # 4 token chunks
CCH = C // P     # 8 channel chunks
HC = HID // P    # 32 hidden chunks
PAIRS = H // 2   # 8 head pairs
EPS = 1e-5
N_CORES = 8
REPLICA_GROUPS = [[0, 1], [2, 3], [4, 5], [6, 7]]

_cache = {}


def _layernorm(nc, pools, x_state, out_bf, g_tile, b_tile, stats_in=None):
    """out_bf[P,NT,C] (bf16) = LN(x_state[P,NT,C] fp32) over C.

    stats_in: optional [P,NT,2,6] bn_stats tile already computed by the
    producer of x_state (pipelined off the critical path)."""
    sb, ps = pools["sb"], pools["ps"]
    eps_t = pools["eps"]
    mv = sb.tile([P, NT, 2], F32, tag="lnmv", bufs=4, name="lnmv")
    for t in range(NT):
        if stats_in is None:
            stats = sb.tile([P, 2, 6], F32, tag="lnstats", bufs=4,
                            name=f"lnst{t}")
            xg = x_state[:, t, :].rearrange("p (b f) -> p b f", f=512)
            for g in range(2):
                nc.vector.bn_stats(stats[:, g, :], xg[:, g, :])
        else:
            stats = stats_in[:, t, :, :]
        nc.vector.bn_aggr(mv[:, t, :], stats[:])
    # rstd = 1/sqrt(var+eps): Sqrt on ACT (one batched instr; sqrt table),
    # exact reciprocal on DVE.  Avoids the Ln/Exp table round-trips.
    std = sb.tile([P, NT], F32, tag="lnstd", bufs=4, name="lnstd")
    rstd = sb.tile([P, NT], F32, tag="lnrstd", bufs=4, name="lnrstd")
    nc.scalar.activation(std[:], mv[:, :, 1:2], AF.Sqrt, bias=eps_t[:])
    nc.vector.reciprocal(rstd[:], std[:])
    for t in range(NT):
        # split normalize across DVE and GpSimd so the block-boundary
        # latency halves (both engines are otherwise idle here)
        eng = nc.vector if t < 2 else nc.gpsimd
        eng.tensor_scalar(
            out=out_bf[:, t, :], in0=x_state[:, t, :],
            scalar1=mv[:, t, 0:1], scalar2=rstd[:, t:t + 1],
            op0=ALU.subtract, op1=ALU.mult)
        if g_tile is not None:
            eng.tensor_mul(out_bf[:, t, :], out_bf[:, t, :], g_tile[:])
        if b_tile is not None:
            eng.tensor_add(out_bf[:, t, :], out_bf[:, t, :], b_tile[:])


def _transpose(nc, pools, src_bf, dst_bf, id_bf):
    """dst_bf[P,CCH,N] = transpose of src_bf[P,NT,C] (channel-major).

    XBAR DMA transpose: off the PE and DVE entirely (4 descriptors,
    spread over both HWDGE queues)."""
    for t in range(NT):
        eng = nc.sync if t % 2 == 0 else nc.scalar
        eng.dma_start_transpose(
            dst_bf[:, :, t * P:(t + 1) * P], src_bf[:, t, :])


def _attention(nc, pools, qT, kvT, kv_nat, ot, consts, self_mode):
    """ot[P,CCH,N] (bf16) = per-head softmax(qk/8) @ v, heads = channel dim.

    qT/kvT: [P,CCH,N] bf16 transposed normed activations (channel on part.)
    kv_nat: [P,NT,C]  bf16 normed activations (token on partitions)
    """
    sb, ps = pools["sb"], pools["ps"]
    id_f32 = consts["id_f32"]

    r_all = None
    ps_rt = None
    if self_mode:
        r_all = sb.tile([P, NT, H], F32, tag="r_all", bufs=2, name="r_all")
    else:
        ps_rt = ps.tile([16, N], F32, tag="ps_rt", bufs=1, name="ps_rt")
    rt = sb.tile([16, N], F32R, tag="rt", bufs=2, name="rt")

    n_denom = 0
    for j in range(PAIRS):
        ha, hb = 2 * j, 2 * j + 1
        e_a, e_b = [], []
        # scores S^T chunks + exp (row-packed head pair)
        for sc in range(NT):
            ssl = slice(sc * P, (sc + 1) * P)
            psa = ps.tile([P, N], F32, tag="ps_short", bufs=3,
                          name=f"psa{j}_{sc}")
            psb = ps.tile([P, N], F32, tag="ps_short", bufs=3,
                          name=f"psb{j}_{sc}")
            nc.tensor.matmul(psa[:], lhsT=kvT[0:64, j, ssl],
                             rhs=qT[0:64, j, :], start=True, stop=True,
                             tile_position=(0, 0))
            nc.tensor.matmul(psb[:], lhsT=kvT[64:128, j, ssl],
                             rhs=qT[64:128, j, :], start=True, stop=True,
                             tile_position=(64, 0))
            ea = sb.tile([P, N], BF16, tag="eh", bufs=32, name=f"ea{j}_{sc}")
            eb = sb.tile([P, N], BF16, tag="eh", bufs=32, name=f"eb{j}_{sc}")
            if self_mode:
                # symmetric E: free-dim accumulation gives the softmax denom
                nc.scalar.activation(ea[:], psa[:], AF.Exp, scale=0.125,
                                     accum_out=r_all[:, sc, ha:ha + 1])
                nc.scalar.activation(eb[:], psb[:], AF.Exp, scale=0.125,
                                     accum_out=r_all[:, sc, hb:hb + 1])
            else:
                nc.scalar.activation(ea[:], psa[:], AF.Exp, scale=0.125)
                nc.scalar.activation(eb[:], psb[:], AF.Exp, scale=0.125)
            e_a.append(ea)
            e_b.append(eb)

        if not self_mode:
            # denominators: rows of ps_rt accumulate sum_s E^T[s, n] per head
            sel = consts["sel"]
            for sc in range(NT):
                for hh, ee in ((ha, e_a[sc]), (hb, e_b[sc])):
                    nc.tensor.matmul(
                        ps_rt[:], lhsT=sel[:, hh, :], rhs=ee[:],
                        start=(n_denom == 0),
                        stop=(n_denom == 2 * PAIRS * NT - 1),
                        tile_position=(0, 0))
                    n_denom += 1

        # AV: U^T accumulate over s chunks, col-packed head pair
        psu = ps.tile([P, N], F32, tag="ps_acc", bufs=4, name=f"psu{j}")
        for sc in range(NT):
            nc.tensor.matmul(psu[0:64, :],
                             lhsT=kv_nat[:, sc, ha * D:(ha + 1) * D],
                             rhs=e_a[sc][:], start=(sc == 0),
                             stop=(sc == NT - 1), tile_position=(0, 0))
            nc.tensor.matmul(psu[64:128, :],
                             lhsT=kv_nat[:, sc, hb * D:(hb + 1) * D],
                             rhs=e_b[sc][:], start=(sc == 0),
                             stop=(sc == NT - 1), tile_position=(0, 64))
        # unnormalized U^T into the output tile (bf16)
        nc.vector.tensor_copy(ot[:, j, :], psu[:])

    # reciprocal denominators, laid out [16 heads, N]
    if self_mode:
        for sc in range(NT):
            pst = ps.tile([16, P], F32, tag="ps_short", bufs=3,
                          name=f"psrt{sc}")
            nc.tensor.transpose(pst[:], r_all[:, sc, :], id_f32[:])
            nc.vector.tensor_copy(rt[:, sc * P:(sc + 1) * P], pst[:])
        with nc.allow_low_precision(reason="softmax denom recip in f32r"):
            nc.vector.reciprocal(rt[:], rt[:])
    else:
        with nc.allow_low_precision(reason="softmax denom recip in f32r"):
            nc.vector.reciprocal(rt[:], ps_rt[:])

    # normalize: broadcast recip rows over head partitions via K=16 matmul
    bmat = consts["bmat"]
    for j in range(PAIRS):
        psc = ps.tile([P, N], F32, tag="ps_short", bufs=3, name=f"psbc{j}")
        nc.tensor.matmul(psc[:], lhsT=bmat[:, j * P:(j + 1) * P],
                         rhs=rt[:], start=True, stop=True,
                         tile_position=(0, 0))
        nc.vector.tensor_mul(ot[:, j, :], ot[:, j, :], psc[:])


def _proj_residual(nc, pools, ot, w_sb, x_state, bias_tile, stats_out=None):
    """x_state += ot.T @ w  (w_sb: [P,CCH,C] bf16).

    stats_out: optional [P,NT,2,6] tile; bn_stats for the *next* LN is
    issued right after each residual half finalizes (hides LN latency)."""
    ps = pools["ps"]
    for t in range(NT):
        for co in range(2):
            cosl = slice(co * 512, (co + 1) * 512)
            psm = ps.tile([P, 512], F32, tag="ps_acc", bufs=4,
                          name=f"pspj{t}_{co}")
            for c in range(CCH):
                nc.tensor.matmul(psm[:], lhsT=ot[:, c, t * P:(t + 1) * P],
                                 rhs=w_sb[:, c, cosl], start=(c == 0),
                                 stop=(c == CCH - 1))
            nc.vector.tensor_add(x_state[:, t, cosl], x_state[:, t, cosl],
                                 psm[:])
            if bias_tile is not None:
                nc.vector.tensor_add(x_state[:, t, cosl],
                                     x_state[:, t, cosl], bias_tile[:, cosl])
            if stats_out is not None:
                nc.vector.bn_stats(stats_out[:, t, co, :],
                                   x_state[:, t, cosl])


def _mlp(nc, pools, x_state, consts, flags, stats_in=None, stats_out=None):
    """x_state += fc2(gelu(fc1(LN2(x_state)))).

    stats_in: pipelined bn_stats for LN2 (from _proj_residual).
    stats_out: bn_stats for the next block's LN1, issued as fc2 halves land."""
    sb, ps = pools["sb"], pools["ps"]
    x2n = sb.tile([P, NT, C], BF16, tag="n_bf", bufs=2, name="x2n")
    _layernorm(nc, pools, x_state, x2n, consts.get("g2t"), consts.get("b2t"),
               stats_in=stats_in)
    x2T = sb.tile([P, CCH, N], BF16, tag="nT", bufs=2, name="x2T")
    _transpose(nc, pools, x2n, x2T, consts["id_bf"])

    fc1w, fc2w_dram = consts["fc1w"], consts["fc2w_dram"]
    fc1b = consts.get("fc1bt")
    hacts = []
    for ht in range(HC):
        psh = ps.tile([P, N], F32, tag="ps_acc", bufs=4, name=f"psh{ht}")
        for c in range(CCH):
            nc.tensor.matmul(psh[:], lhsT=fc1w[:, c, ht * P:(ht + 1) * P],
                             rhs=x2T[:, c, :], start=(c == 0),
                             stop=(c == CCH - 1))
        hact = sb.tile([P, N], BF16, tag="eh", bufs=32, name=f"hact{ht}")
        if fc1b is not None:
            nc.scalar.activation(hact[:], psh[:], AF.Gelu,
                                 bias=fc1b[:, ht:ht + 1])
        else:
            nc.scalar.activation(hact[:], psh[:], AF.Gelu)
        hacts.append(hact)

    fc2b = consts.get("fc2bt")
    for co in range(2):
        cosl = slice(co * 512, (co + 1) * 512)
        psms = [ps.tile([P, 512], F32, tag="ps_acc", bufs=4,
                        name=f"psm2_{co}_{t}") for t in range(NT)]
        for hc in range(HC):
            wt = sb.tile([P, 512], BF16, tag="fc2w", bufs=6,
                         name=f"f2w{co}_{hc}")
            nc.sync.dma_start(wt[:], fc2w_dram[hc * P:(hc + 1) * P, cosl])
            for t in range(NT):
                nc.tensor.matmul(psms[t][:],
                                 lhsT=hacts[hc][:, t * P:(t + 1) * P],
                                 rhs=wt[:], start=(hc == 0),
                                 stop=(hc == HC - 1))
        for t in range(NT):
            nc.vector.tensor_add(x_state[:, t, cosl], x_state[:, t, cosl],
                                 psms[t][:])
            if fc2b is not None:
                nc.vector.tensor_add(x_state[:, t, cosl],
                                     x_state[:, t, cosl], fc2b[:, cosl])
            if stats_out is not None:
                nc.vector.bn_stats(stats_out[:, t, co, :],
                                   x_state[:, t, cosl])


def _block(nc, pools, x_state, consts, flags, kv_state=None, stats_in=None,
           stats_out=None):
    """One transformer block.  kv_state=None -> self-attn on x_state.

    stats_in: pipelined bn_stats for this block's LN1 (of x_state).
    stats_out: where to put bn_stats for the next block's LN1."""
    sb = pools["sb"]
    xn = sb.tile([P, NT, C], BF16, tag="n_bf", bufs=2, name="xn")
    _layernorm(nc, pools, x_state, xn, consts.get("g1t"), consts.get("b1t"),
               stats_in=stats_in)
    xnT = sb.tile([P, CCH, N], BF16, tag="nT", bufs=2, name="xnT")
    _transpose(nc, pools, xn, xnT, consts["id_bf"])

    if kv_state is None:
        kv_n, kv_T, self_mode = xn, xnT, True
    else:
        kv_n = sb.tile([P, NT, C], BF16, tag="n_bf", bufs=2, name="pn")
        _layernorm(nc, pools, kv_state, kv_n, consts.get("g1t"),
                   consts.get("b1t"))
        kv_T = sb.tile([P, CCH, N], BF16, tag="nT", bufs=2, name="pnT")
        _transpose(nc, pools, kv_n, kv_T, consts["id_bf"])
        self_mode = False

    ot = sb.tile([P, CCH, N], BF16, tag="ot", bufs=1, name="ot")
    _attention(nc, pools, xnT, kv_T, kv_n, ot, consts, self_mode)
    st2 = sb.tile([P, NT, 2, 6], F32, tag="pstats", bufs=3, name="st2")
    _proj_residual(nc, pools, ot, consts["projw"], x_state,
                   consts.get("projbt"), stats_out=st2)
    _mlp(nc, pools, x_state, consts, flags, stats_in=st2, stats_out=stats_out)


def _build(n_self, flags, exchange=True, reps=1):
    """flags: dict of bools: g1,b1,g2,b2,projb,fc1b,fc2b nontrivial.

    reps>1 repeats the whole computation on its own output (state feedback
    in SBUF) — used only for device-time measurement by chain slope."""
    nc = bacc.Bacc("TRN2", target_bir_lowering=False, debug=False,
                   num_devices=N_CORES)

    own_d = nc.dram_tensor("own", [P, NT, C], F32, kind="ExternalInput").ap()
    projw_d = nc.dram_tensor("projw", [P, CCH, C], BF16,
                             kind="ExternalInput").ap()
    fc1w_d = nc.dram_tensor("fc1w", [P, CCH, HID], BF16,
                            kind="ExternalInput").ap()
    fc2w_d = nc.dram_tensor("fc2w", [HID, C], BF16, kind="ExternalInput").ap()
    idbf_d = nc.dram_tensor("id_bf", [P, P], BF16, kind="ExternalInput").ap()
    idf_d = nc.dram_tensor("id_f32", [P, P], F32, kind="ExternalInput").ap()
    sel_d = nc.dram_tensor("sel", [P, H, H], BF16, kind="ExternalInput").ap()
    bmat_d = nc.dram_tensor("bmat", [16, C], F32R, kind="ExternalInput").ap()
    extra_d = {}
    for nm, shape in (("g1", [C]), ("b1", [C]), ("g2", [C]), ("b2", [C]),
                      ("projb", [C]), ("fc2b", [C])):
        if flags[nm]:
            extra_d[nm] = nc.dram_tensor(nm, shape, F32,
                                         kind="ExternalInput").ap()
    if flags["fc1b"]:
        extra_d["fc1b"] = nc.dram_tensor("fc1b", [P, HC], F32,
                                         kind="ExternalInput").ap()
    out_d = nc.dram_tensor("out", [P, NT, C], F32, kind="ExternalOutput").ap()

    with tile.TileContext(nc) as tc:
        with tc.tile_pool(name="sb", bufs=1) as sb, \
             tc.tile_pool(name="ps", bufs=1, space="PSUM") as ps, \
             tc.tile_pool(name="dram", bufs=1, space="DRAM") as dram:
            pools = {"sb": sb, "ps": ps, "dram": dram}
            eps_t = sb.tile([P, 1], F32, tag="eps", name="eps_t")
            nc.vector.memset(eps_t[:], EPS)
            pools["eps"] = eps_t

            # persistent state + constants first (they gate the first
            # block's LN/transpose/attention), big weights after (projw is
            # needed at proj time, fc1w only at MLP time).
            x_state = sb.tile([P, NT, C], F32, tag="x_state", name="x_state")
            nc.sync.dma_start(x_state[:], own_d)
            id_bf = sb.tile([P, P], BF16, tag="id_bf", name="id_bf")
            nc.sync.dma_start(id_bf[:], idbf_d)
            id_f32 = sb.tile([P, P], F32, tag="id_f32", name="id_f32")
            nc.sync.dma_start(id_f32[:], idf_d)
            sel = sb.tile([P, H, H], BF16, tag="sel", name="sel")
            nc.sync.dma_start(sel[:], sel_d)
            bmat = sb.tile([16, C], F32R, tag="bmat", name="bmat")
            nc.sync.dma_start(bmat[:], bmat_d)
            projw = sb.tile([P, CCH, C], BF16, tag="projw", name="projw")
            nc.sync.dma_start(projw[:], projw_d)
            fc1w = sb.tile([P, CCH, HID], BF16, tag="fc1w", name="fc1w")
            nc.sync.dma_start(fc1w[:], fc1w_d)

            consts = {"id_bf": id_bf, "id_f32": id_f32, "sel": sel,
                      "bmat": bmat, "projw": projw, "fc1w": fc1w,
                      "fc2w_dram": fc2w_d}
            # optional gain/bias tiles
            for nm, key in (("g1", "g1t"), ("b1", "b1t"), ("g2", "g2t"),
                            ("b2", "b2t"), ("projb", "projbt"),
                            ("fc2b", "fc2bt")):
                if flags[nm]:
                    t_ = sb.tile([P, C], F32, tag=nm, name=nm + "t")
                    nc.sync.dma_start(t_[:],
                                      extra_d[nm].to_broadcast((P, C)))
                    consts[key] = t_
            if flags["fc1b"]:
                t_ = sb.tile([P, HC], F32, tag="fc1b", name="fc1bt")
                nc.sync.dma_start(t_[:], extra_d["fc1b"])
                consts["fc1bt"] = t_

            stats = None
            for r in range(reps):
                for i in range(n_self):
                    nxt = sb.tile([P, NT, 2, 6], F32, tag="pstats", bufs=3,
                                  name=f"st_{r}_{i}")
                    _block(nc, pools, x_state, consts, flags,
                           stats_in=stats, stats_out=nxt)
                    stats = nxt

                # exchange states within the pair, then cross-attn block
                partner = sb.tile([P, NT, C], F32, tag="partner",
                                  name=f"partner{r}")
                if exchange:
                    b_in = dram.tile([P, NT, C], F32, name=f"b_in{r}")
                    b_out = dram.tile([P, NT, C], F32, name=f"b_out{r}")
                    nc.sync.dma_start(b_in[:], x_state[:])
                    nc.gpsimd.collective_compute(
                        "AllReduce", ALU.add, replica_groups=REPLICA_GROUPS,
                        ins=[b_in.opt()], outs=[b_out.opt()])
                    nc.sync.dma_start(partner[:], b_out[:])
                    nc.vector.tensor_sub(partner[:], partner[:], x_state[:])
                else:
                    nc.vector.tensor_copy(partner[:], x_state[:])

                nxt = (sb.tile([P, NT, 2, 6], F32, tag="pstats", bufs=3,
                               name=f"stc_{r}")
                       if r < reps - 1 else None)
                _block(nc, pools, x_state, consts, flags, kv_state=partner,
                       stats_in=stats, stats_out=nxt)
                stats = nxt

            nc.sync.dma_start(out_d, x_state[:])
    nc.compile()
    return nc


def _get_nc(n_self, flags):
    key = (n_self, tuple(sorted(flags.items())))
    if key not in _cache:
        _cache[key] = _build(n_self, flags)
    return _cache[key]


def _nontrivial(a, val=0.0):
    return not np.allclose(np.asarray(a, np.float32), val, atol=0.0, rtol=0.0)


def kernel(**inputs):
    x = np.ascontiguousarray(np.asarray(inputs["x"], np.float32))
    y = np.ascontiguousarray(np.asarray(inputs["y"], np.float32))
    n1g, n1b = inputs["norm1_g"], inputs["norm1_b"]
    n2g, n2b = inputs["norm2_g"], inputs["norm2_b"]
    proj_w, proj_b = inputs["proj_w"], inputs["proj_b"]
    fc1_w, fc1_b = inputs["fc1_w"], inputs["fc1_b"]
    fc2_w, fc2_b = inputs["fc2_w"], inputs["fc2_b"]
    is_selfatt = int(np.asarray(inputs["is_selfatt"]))

    flags = {
        "g1": _nontrivial(n1g, 1.0), "b1": _nontrivial(n1b),
        "g2": _nontrivial(n2g, 1.0), "b2": _nontrivial(n2b),
        "projb": _nontrivial(proj_b), "fc1b": _nontrivial(fc1_b),
        "fc2b": _nontrivial(fc2_b),
    }
    n_self = 4 if is_selfatt else 0
    nc = _get_nc(n_self, flags)

    bf = ml_dtypes.bfloat16
    projw_h = np.ascontiguousarray(
        np.asarray(proj_w, np.float32).reshape(CCH, P, C).transpose(1, 0, 2)
    ).astype(bf)
    fc1w_h = np.ascontiguousarray(
        np.asarray(fc1_w, np.float32).reshape(CCH, P, HID).transpose(1, 0, 2)
    ).astype(bf)
    fc2w_h = np.ascontiguousarray(np.asarray(fc2_w, np.float32)).astype(bf)
    id_h = np.eye(P, dtype=np.float32)
    sel_h = np.zeros((P, H, H), np.float32)
    sel_h[:, np.arange(H), np.arange(H)] = 1.0
    sel_h = sel_h.astype(bf)
    bmat_h = np.zeros((16, C), np.float32)
    for j in range(PAIRS):
        bmat_h[2 * j, j * P:j * P + 64] = 1.0
        bmat_h[2 * j + 1, j * P + 64:(j + 1) * P] = 1.0

    base = {
        "projw": projw_h, "fc1w": fc1w_h, "fc2w": fc2w_h,
        "id_bf": id_h.astype(bf), "id_f32": id_h,
        "sel": sel_h, "bmat": bmat_h,
    }
    for nm, arr in (("g1", n1g), ("b1", n1b), ("g2", n2g), ("b2", n2b),
                    ("projb", proj_b), ("fc2b", fc2_b)):
        if flags[nm]:
            base[nm] = np.ascontiguousarray(np.asarray(arr, np.float32))
    if flags["fc1b"]:
        base["fc1b"] = np.ascontiguousarray(
            np.asarray(fc1_b, np.float32).reshape(HC, P).T)

    in_maps = []
    for core in range(N_CORES):
        bidx = core // 2
        own = x[bidx] if core % 2 == 0 else y[bidx]
        own_dev = np.ascontiguousarray(
            own.reshape(NT, P, C).transpose(1, 0, 2))
        m = dict(base)
        m["own"] = own_dev
        in_maps.append(m)

    res = run_bass_kernel_spmd(nc, in_maps, core_ids=list(range(N_CORES)))

    def unpack(core):
        o = np.asarray(res.results[core]["out"], np.float32)
        return o.transpose(1, 0, 2).reshape(N, C)

    x1 = np.stack([unpack(2 * b) for b in range(B)])
    y1 = np.stack([unpack(2 * b + 1) for b in range(B)])
    return (x1, y1)

